# revision 22
# baseline (speedup 1.0000x reference)
"""Trainium2 Bass kernel for nn_Capsule (Efficient-CapsNet style capsule layer).

Math (see reference):
    u[b,k,j,:] = x[b,j,:] @ w[k,j,:,:]            # per-(k,j) 16x16 projection
    t[b,k,:]   = sum_j u[b,k,j,:]
    l[b,k,j]   = <u[b,k,j,:], t[b,k,:]> / sqrt(D)
    c          = softmax_k(l) + bias
    s[b,k,:]   = sum_j c[b,k,j] u[b,k,j,:]
    out        = squash(s)

Sharding: the j (N=2048) contraction axis is split over 8 cores (256 j each),
so each core reads only its w slice once (4 MB in bf16).  Cross-core coupling
is a single 64 KB AllReduce of t; the softmax over k is core-local.  Per-core
partial s [32,512] are summed on host, followed by the (tiny) squash.

Key layout trick vs the previous version: w columns are packed (z,k) with k
innermost (col = z*32 + k).  Then the softmax weights c (which vary over k
but not z) broadcast over the MIDDLE z dim of a [p,(s,z,k)] view, keeping
unit stride innermost, so the c-apply multiply runs at DVE 2x with NO
materialized c replication.  The s j-reduction runs on the PE as a
block-diagonal ones matmul accumulating into PSUM across all quads.

Per-core schedule:
  DMA:     xt+ones first, then w (z,k)-packed + bdx interleaved.
  phase 1: 32 accumulating bf16 t-matmuls -> t_partial[32,(z,k)]
           -> AllReduce(t) -> replicate to t_rep[128,512] + bf16 cast
  u-path:  per octet (8 j): 2 matmuls -> PSUM [128,1024] -> cast to SBUF
           bf16 (ACT for early octets, Pool for late; all emitted before the
           per-quad chains so the engine queues drain them during the
           DMA/AllReduce head).
  phase 2: per quad (16 j, [128, (s=4,z=16,k=32)] views):
           prod = u*t (DVE 2x, t bcast over s)
           z-tree 16->8->4->2 (Pool) -> lg f32 (Pool)
           e = exp(lg/4) (ACT); zq = sum_k e (DVE); rz (DVE); c = e*rz (DVE)
           prod2 = u*c (DVE 2x, c bcast over middle z)
           slot fold 4->2 (DVE) ; 2 ones-matmuls accumulate s in PSUM (PE)
  tail:    s_psum[32,512] -> SBUF -> DMA; host sums cores, squash.
"""

import sys

if "/opt/trn_rl_repo" not in sys.path:
    sys.path.insert(0, "/opt/trn_rl_repo")

import numpy as np
import os

B, N, D_IN = 32, 2048, 16
K, D_OUT = 32, 16
NCORES = 8
NS = N // NCORES          # 256 local j per core
NT = NS // 4              # 64 tiles of 4 j
NT2 = NT // 2             # 32 octets (8 j each)
NQ = NT2 // 2             # 16 quads (16 j each)
KZ = K * D_OUT            # 512
EPS = 1e-20

_CACHE = {}


def _pack_inputs(x, w, b):
    """Per-core host-side marshaling into the DMA-friendly layouts (bf16)."""
    import ml_dtypes
    bf = ml_dtypes.bfloat16
    xr = x.astype(bf).astype(np.float32)      # [B, N, D_IN]
    wr = w.astype(bf).astype(np.float32)      # [K, N, D_IN, D_OUT]
    ones_bd = np.tile(np.eye(B, dtype=np.float32), (4, 1)).astype(bf)  # [128, 32]
    per_core = []
    for r in range(NCORES):
        js, je = r * NS, (r + 1) * NS
        # w_host[64h+q, t2*512 + (z*32+k)] = w[k, js+(2*t2+h)*4+jr, i, z], q=jr*16+i
        wc = wr[:, js:je]                         # [K, NS, D_IN, D_OUT]
        wc = wc.transpose(1, 2, 3, 0)             # [NS, D_IN, D_OUT, K]  (j, i, z, k)
        wc = wc.reshape(NT, 64, KZ)               # [jt, (jr i), (z k)]
        wc = wc.reshape(NT2, 2, 64, KZ).transpose(1, 2, 0, 3)  # [h, q, t2, c]
        w_host = np.ascontiguousarray(wc.reshape(128, NT2 * KZ)).astype(bf)

        # block-diagonal x for the u matmuls (unchanged layout):
        # bdx[64h+q, t2*128 + jr*32 + b] = x[b, j(tile,jr), i] iff q == jr*16+i
        xc = xr[:, js:je, :]                      # [B, NS, D_IN]
        xc = xc.transpose(1, 2, 0)                # [NS, D_IN, B]  (j, i, b)
        bdx = np.zeros((2, 64, NT2, 128), dtype=np.float32)   # [h, q, t2, col]
        xt4 = xc.reshape(NT2, 2, 4, D_IN, B)      # [t2, h, jr, i, b]
        for jr in range(4):
            bdx[:, jr * 16:(jr + 1) * 16, :, jr * 32:(jr + 1) * 32] = (
                xt4[:, :, jr].transpose(1, 2, 0, 3)           # [h, i, t2, b]
            )
        bdx_host = np.ascontiguousarray(bdx.reshape(128, NT2 * 128)).astype(bf)

        # dense xT for the t matmuls: xt[jj*16+i, t2*32+b] = x[b, js+t2*8+jj, i]
        xt = xc.reshape(NT2, 8, D_IN, B)          # [t2, jj, i, b]
        xt = xt.transpose(1, 2, 0, 3)             # [jj, i, t2, b]
        xt_host = np.ascontiguousarray(xt.reshape(128, NT2 * B)).astype(bf)

        per_core.append({"w": w_host, "bdx": bdx_host, "xt": xt_host,
                         "ones": ones_bd})

    if np.any(b):
        # brep[p=(jr*32+bb), tile*K + k] = b[k, j(tile,jr)]  (replicated over bb)
        for r in range(NCORES):
            js = r * NS
            bc = b[:, js:js + NS, 0]                         # [K, NS]
            br = bc.transpose(1, 0).reshape(NT, 4, 1, K)     # [tile, jr, 1, k]
            br = np.broadcast_to(br, (NT, 4, 32, K))         # replicate over batch
            brep = br.transpose(1, 2, 0, 3).reshape(128, NT * K)
            per_core[r]["brep"] = np.ascontiguousarray(brep, dtype=np.float32)
        with_bias = True
    else:
        with_bias = False
    return per_core, with_bias


def _build(with_bias, sim_collective=False):
    from concourse import bacc, mybir
    from concourse.tile import TileContext

    f32 = mybir.dt.float32
    bf16d = mybir.dt.bfloat16

    nc = bacc.Bacc("TRN2", target_bir_lowering=False, debug=False,
                   num_devices=NCORES)
    w_in = nc.declare_dram_parameter("w", [128, NT2 * KZ], bf16d, isOutput=False)
    bdx_in = nc.declare_dram_parameter("bdx", [128, NT2 * 128], bf16d, isOutput=False)
    xt_in = nc.declare_dram_parameter("xt", [128, NT2 * B], bf16d, isOutput=False)
    ones_in = nc.declare_dram_parameter("ones", [128, B], bf16d, isOutput=False)
    brep_in = None
    if with_bias:
        brep_in = nc.declare_dram_parameter("brep", [128, NT * K], f32, isOutput=False)
    s_out = nc.declare_dram_parameter("s_part", [32, KZ], f32, isOutput=True)

    t_ar_in = nc.dram_tensor("t_ar_in", [32, KZ], f32)
    t_ar_out = nc.dram_tensor("t_ar_out", [32, KZ], f32, addr_space="Shared")

    with TileContext(nc) as tc:
        with (
            tc.tile_pool(name="wp", bufs=1) as wp,
            tc.tile_pool(name="xp", bufs=1) as xp,
            tc.tile_pool(name="ubp", bufs=1) as ubp,
            tc.tile_pool(name="sp", bufs=1) as sp,
            tc.tile_pool(name="work", bufs=2) as work,
            tc.tile_pool(name="zp", bufs=2) as zp,
            tc.tile_pool(name="small", bufs=4) as small,
            tc.tile_pool(name="pu", bufs=3, space="PSUM") as pu,
            tc.tile_pool(name="pt", bufs=1, space="PSUM") as pt,
            tc.tile_pool(name="psm", bufs=1, space="PSUM") as psm,
        ):
            bf16 = mybir.dt.bfloat16
            # ---- input DMAs: t-path first (xt, w), bdx interleaved so the
            # u-matmul/cast pipeline can start early; ones is tiny. ----
            w_sb = wp.tile([128, NT2 * KZ], bf16, tag="w")
            bdx_sb = xp.tile([128, NT2 * 128], bf16, tag="bdx")
            wq = NT2 * KZ // 8
            bq4 = NT2 * 128 // 4
            nc.sync.dma_start(out=w_sb[:, 0:wq], in_=w_in[:, 0:wq])
            xt_sb = xp.tile([128, NT2 * B], bf16, tag="xt")
            nc.sync.dma_start(out=xt_sb[:, :], in_=xt_in[:, :])
            ones_sb = xp.tile([128, B], bf16, tag="ones")
            nc.sync.dma_start(out=ones_sb[:, :], in_=ones_in[:, :])
            nc.sync.dma_start(out=bdx_sb[:, 0:bq4], in_=bdx_in[:, 0:bq4])
            for ci in range(1, 8):
                nc.sync.dma_start(out=w_sb[:, ci * wq:(ci + 1) * wq],
                                  in_=w_in[:, ci * wq:(ci + 1) * wq])

            brep_sb = None
            if with_bias:
                brep_sb = xp.tile([128, NT * K], f32, tag="brep")
                nc.sync.dma_start(out=brep_sb[:, :], in_=brep_in[:, :])

            # ---- phase 1: partial t ([32, (z,k)] psum, accumulated);
            # u-matmuls+casts for the first 8 octets ride along so the ACT
            # queue has cast work during the head (PE keeps up at full
            # p-state). ----
            u_bf = ubp.tile([128, NT2 * 1024], bf16, tag="u_bf")

            def emit_octet(o, cast_eng):
                uo = pu.tile([128, 1024], f32, tag="u")
                for h in range(2):
                    nc.tensor.matmul(uo[:, h * KZ:(h + 1) * KZ],
                                     bdx_sb[64 * h:64 * h + 64,
                                            o * 128:(o + 1) * 128],
                                     w_sb[64 * h:64 * h + 64,
                                          o * KZ:(o + 1) * KZ],
                                     start=True, stop=True)
                # GPSIMD cannot access PSUM on real HW: casts are ACT-only
                nc.scalar.copy(u_bf[:, o * 1024:(o + 1) * 1024], uo[:, :])

            t_ps = pt.tile([32, KZ], f32, tag="t")
            for t2 in range(NT2):
                nc.tensor.matmul(t_ps[0:32, 0:KZ],
                                 xt_sb[:, t2 * B:(t2 + 1) * B],
                                 w_sb[:, t2 * KZ:(t2 + 1) * KZ],
                                 start=(t2 == 0), stop=(t2 == NT2 - 1))
                if t2 < 8:
                    emit_octet(t2, "act")
            # t_loc copy on DVE: it is idle pre-AllReduce, and this keeps the
            # ACT queue free for the u casts.
            t_loc = sp.tile([32, KZ], f32, tag="t_loc")
            nc.vector.tensor_copy(t_loc[:, :], t_ps[0:32, 0:KZ])
            nc.sync.dma_start(out=t_ar_in[:, :], in_=t_loc[:, :])
            if sim_collective:
                # timing-model stand-in for the AllReduce (single-core sim)
                nc.gpsimd.dma_start(out=t_ar_out[:, :], in_=t_ar_in[:, :])
            else:
                nc.gpsimd.collective_compute(
                    "AllReduce",
                    mybir.AluOpType.add,
                    replica_groups=[list(range(NCORES))],
                    ins=[t_ar_in[:, :].opt()],
                    outs=[t_ar_out[:, :].opt()],
                )
            # late bdx quarters ride the window left idle by the AllReduce
            for ci in range(1, 4):
                nc.sync.dma_start(out=bdx_sb[:, ci * bq4:(ci + 1) * bq4],
                                  in_=bdx_in[:, ci * bq4:(ci + 1) * bq4])
            # replicate t to all 4 jr partition groups: one DMA + partition
            # doubling (DVE is idle at this point; a single broadcast-read
            # DMA would degenerate to per-element descriptors)
            t_rep = sp.tile([128, KZ], f32, tag="t_rep")
            nc.sync.dma_start(out=t_rep[0:32, :], in_=t_ar_out[:, :])
            nc.vector.tensor_copy(t_rep[32:64, :], t_rep[0:32, :])
            nc.vector.tensor_copy(t_rep[64:128, :], t_rep[0:64, :])
            t_rep_bf = sp.tile([128, KZ], bf16, tag="t_rep_bf")
            nc.scalar.copy(t_rep_bf[:, :], t_rep[:, :])

            t_bc = (t_rep_bf[:, :]
                    .rearrange("p (z k) -> p z k", k=K)
                    .unsqueeze(1).to_broadcast((128, 4, D_OUT, K)))

            s_ps = psm.tile([32, KZ], f32, tag="s")

            # ---- phase 2: per quad; u-matmuls+casts for octets 8..31
            # stream through the loop (ACT for 8..23, Pool for 24..31),
            # always >= 4 quads ahead of their consumer ----
            for q in range(NQ):
                for o in (8 + 2 * q, 9 + 2 * q):
                    if o < NT2:
                        emit_octet(o, "act")
                uq = (u_bf[:, q * 2048:(q + 1) * 2048]
                      .rearrange("p (s z k) -> p s z k", s=4, k=K))
                # prod = u * t  (DVE 2x; t broadcast over the 4 slots)
                P1 = work.tile([128, 2048], bf16, tag="P1")
                nc.vector.tensor_tensor(
                    P1[:, :].rearrange("p (s z k) -> p s z k", s=4, k=K),
                    uq, t_bc, op=mybir.AluOpType.mult)
                # z-tree 16->8->4->2 on Pool (k stays innermost unit stride)
                p1v = P1[:, :].rearrange("p (s z k) -> p s z k", s=4, k=K)
                Z1 = zp.tile([128, 1024], bf16, tag="Z1")
                z1v = Z1[:, :].rearrange("p (s z k) -> p s z k", s=4, k=K)
                nc.gpsimd.tensor_tensor(z1v, p1v[:, :, 0:8], p1v[:, :, 8:16],
                                        op=mybir.AluOpType.add)
                Z2 = zp.tile([128, 512], bf16, tag="Z2")
                z2v = Z2[:, :].rearrange("p (s z k) -> p s z k", s=4, k=K)
                nc.gpsimd.tensor_tensor(z2v, z1v[:, :, 0:4], z1v[:, :, 4:8],
                                        op=mybir.AluOpType.add)
                Z3 = zp.tile([128, 256], bf16, tag="Z3")
                z3v = Z3[:, :].rearrange("p (s z k) -> p s z k", s=4, k=K)
                nc.gpsimd.tensor_tensor(z3v, z2v[:, :, 0:2], z2v[:, :, 2:4],
                                        op=mybir.AluOpType.add)
                lg = small.tile([128, 4 * K], f32, tag="lg")
                nc.gpsimd.tensor_tensor(
                    lg[:, :].rearrange("p (s k) -> p s k", k=K),
                    z3v[:, :, 0, :], z3v[:, :, 1, :], op=mybir.AluOpType.add)
                # softmax over k (per slot)
                e = small.tile([128, 4 * K], f32, tag="e")
                nc.scalar.activation(e[:, :], lg[:, :],
                                     mybir.ActivationFunctionType.Exp,
                                     scale=0.25)
                zq = small.tile([128, 4], f32, tag="zq")
                nc.vector.tensor_reduce(
                    zq[:, :],
                    e[:, :].rearrange("p (s k) -> p s k", k=K),
                    axis=mybir.AxisListType.X, op=mybir.AluOpType.add)
                rz = small.tile([128, 4], f32, tag="rz")
                nc.vector.reciprocal(rz[:, :], zq[:, :])
                c = small.tile([128, 4 * K], bf16, tag="c")
                rz_bc = rz[:, :].unsqueeze(-1).to_broadcast((128, 4, K))
                nc.gpsimd.tensor_tensor(
                    c[:, :].rearrange("p (s k) -> p s k", k=K),
                    e[:, :].rearrange("p (s k) -> p s k", k=K),
                    rz_bc, op=mybir.AluOpType.mult)
                if with_bias:
                    nc.vector.tensor_tensor(
                        c[:, :], c[:, :],
                        brep_sb[:, q * 4 * K:(q + 1) * 4 * K],
                        op=mybir.AluOpType.add)

                # prod2 = u * c (c broadcast over the middle z dim); slots
                # 0-2 on DVE (2x), slot 3 on Pool to balance the engines
                c_bc3 = (c[:, 0:3 * K].rearrange("p (s k) -> p s k", k=K)
                         .unsqueeze(2).to_broadcast((128, 3, D_OUT, K)))
                P2 = work.tile([128, 2048], bf16, tag="P2")
                nc.vector.tensor_tensor(
                    P2[:, 0:1536].rearrange("p (s z k) -> p s z k", s=3, k=K),
                    uq[:, 0:3], c_bc3, op=mybir.AluOpType.mult)
                c_s3 = (c[:, 3 * K:4 * K].unsqueeze(1)
                        .to_broadcast((128, D_OUT, K)))
                nc.gpsimd.tensor_tensor(
                    P2[:, 1536:2048].rearrange("p (z k) -> p z k", k=K),
                    uq[:, 3], c_s3, op=mybir.AluOpType.mult)
                # slot fold 4->2 (DVE 2x), then two PE ones-matmuls
                # accumulate s[b,(z,k)] in PSUM across all quads
                SH = work.tile([128, 1024], bf16, tag="SH")
                p2v = P2[:, :].rearrange("p (s c) -> p s c", s=4)
                nc.vector.tensor_tensor(
                    SH[:, :].rearrange("p (s c) -> p s c", s=2),
                    p2v[:, 0:2], p2v[:, 2:4], op=mybir.AluOpType.add)
                nc.tensor.matmul(s_ps[0:32, 0:KZ], ones_sb[:, 0:B],
                                 SH[:, 0:KZ],
                                 start=(q == 0), stop=False)
                nc.tensor.matmul(s_ps[0:32, 0:KZ], ones_sb[:, 0:B],
                                 SH[:, KZ:2 * KZ],
                                 start=False, stop=(q == NQ - 1))

            # ---- tail ----
            s_sb = sp.tile([32, KZ], f32, tag="s_sb")
            nc.scalar.copy(s_sb[:, :], s_ps[0:32, 0:KZ])
            nc.sync.dma_start(out=s_out[:, :], in_=s_sb[:, :])

    nc.compile()
    return nc


def _get_nc(with_bias):
    key = ("nc", with_bias)
    if key not in _CACHE:
        _CACHE[key] = _build(with_bias)
    return _CACHE[key]


def _get_runner(with_bias):
    """Build (once) a cached shard_map-jitted executable for the 8-core SPMD
    kernel, mirroring bass2jax.run_bass_via_pjrt but reusable across calls."""
    key = ("runner", with_bias)
    if key in _CACHE:
        return _CACHE[key]

    import jax
    from jax.sharding import Mesh, PartitionSpec
    from jax.experimental.shard_map import shard_map
    from concourse import mybir
    from concourse import bass2jax
    from concourse.bass2jax import (_bass_exec_p, install_neuronx_cc_hook,
                                    partition_id_tensor)

    install_neuronx_cc_hook()
    nc = _get_nc(with_bias)

    partition_name = nc.partition_id_tensor.name if nc.partition_id_tensor else None
    in_names, out_names, out_avals, zero_shapes = [], [], [], []
    for alloc in nc.m.functions[0].allocations:
        if not isinstance(alloc, mybir.MemoryLocationSet):
            continue
        name = alloc.memorylocations[0].name
        if alloc.kind == "ExternalInput":
            if name != partition_name:
                in_names.append(name)
        elif alloc.kind == "ExternalOutput":
            out_names.append(name)
            shape = tuple(alloc.tensor_shape)
            dtype = mybir.dt.np(alloc.dtype)
            out_avals.append(jax.core.ShapedArray(shape, dtype))
            zero_shapes.append((shape, dtype))
    n_params = len(in_names)
    n_outs = len(out_avals)
    all_in_names = list(in_names) + list(out_names)
    if partition_name is not None:
        all_in_names.append(partition_name)

    def _body(*args):
        operands = list(args)
        if partition_name is not None:
            operands.append(partition_id_tensor())
        outs = _bass_exec_p.bind(
            *operands,
            out_avals=tuple(out_avals),
            in_names=tuple(all_in_names),
            out_names=tuple(out_names),
            lowering_input_output_aliases=(),
            sim_require_finite=True,
            sim_require_nnan=True,
            nc=nc,
        )
        return tuple(outs)

    devices = jax.devices()[:NCORES]
    mesh = Mesh(np.asarray(devices), ("core",))
    in_specs = (PartitionSpec("core"),) * (n_params + n_outs)
    out_specs = (PartitionSpec("core"),) * n_outs
    donate = tuple(range(n_params, n_params + n_outs))
    sharded = jax.jit(
        shard_map(_body, mesh=mesh, in_specs=in_specs, out_specs=out_specs,
                  check_rep=False),
        donate_argnums=donate, keep_unused=True)

    def run(per_core):
        concat_in = [
            np.concatenate([np.asarray(per_core[c][nm]) for c in range(NCORES)], axis=0)
            for nm in in_names
        ]
        concat_zeros = [np.zeros((NCORES * sh[0], *sh[1:]), dt)
                        for sh, dt in zero_shapes]
        out_arrs = sharded(*concat_in, *concat_zeros)
        return [
            {nm: np.asarray(out_arrs[i]).reshape(NCORES, *out_avals[i].shape)[c]
             for i, nm in enumerate(out_names)}
            for c in range(NCORES)
        ]

    _CACHE[key] = run
    return run


def kernel(x, w, b, _run_kwargs=None):
    x = np.asarray(x, dtype=np.float32)
    w = np.asarray(w, dtype=np.float32)
    b = np.asarray(b, dtype=np.float32)

    per_core, with_bias = _pack_inputs(x, w, b)
    results = _get_runner(with_bias)(per_core)

    s = np.zeros((B, KZ), dtype=np.float64)
    for r in range(NCORES):
        s += results[r]["s_part"].astype(np.float64)
    # cols are (z,k): s[b, z*32+k] -> [B, K, D_OUT]
    s = s.reshape(B, D_OUT, K).transpose(0, 2, 1)
    s = np.ascontiguousarray(s).astype(np.float32)

    # efficient squash (host-side finalization of the gathered partials)
    n = np.linalg.norm(s.astype(np.float64), axis=-1, keepdims=True)
    out = (1.0 - 1.0 / (np.exp(n) + EPS)) * (s / (n + EPS))
    return out.astype(np.float32)


# revision 25
# speedup vs baseline: 1.0587x; 1.0587x over previous
"""Trainium2 Bass kernel for nn_Capsule (Efficient-CapsNet style capsule layer).

Math (see reference):
    u[b,k,j,:] = x[b,j,:] @ w[k,j,:,:]            # per-(k,j) 16x16 projection
    t[b,k,:]   = sum_j u[b,k,j,:]
    l[b,k,j]   = <u[b,k,j,:], t[b,k,:]> / sqrt(D)
    c          = softmax_k(l) + bias
    s[b,k,:]   = sum_j c[b,k,j] u[b,k,j,:]
    out        = squash(s)

Sharding: the j (N=2048) contraction axis is split over 8 cores (256 j each),
so each core reads only its w slice once (4 MB in bf16).  Cross-core coupling
is a single 64 KB AllReduce of t; the softmax over k is core-local.  Per-core
partial s [32,512] are summed on host, followed by the (tiny) squash.

Key layout trick vs the previous version: w columns are packed (z,k) with k
innermost (col = z*32 + k).  Then the softmax weights c (which vary over k
but not z) broadcast over the MIDDLE z dim of a [p,(s,z,k)] view, keeping
unit stride innermost, so the c-apply multiply runs at DVE 2x with NO
materialized c replication.  The s j-reduction runs on the PE as a
block-diagonal ones matmul accumulating into PSUM across all quads.

Per-core schedule:
  DMA:     xt+ones first, then w (z,k)-packed + bdx interleaved.
  phase 1: 32 accumulating bf16 t-matmuls -> t_partial[32,(z,k)]
           -> AllReduce(t) -> replicate to t_rep[128,512] + bf16 cast
  u-path:  per octet (8 j): 2 matmuls -> PSUM [128,1024] -> cast to SBUF
           bf16 (ACT for early octets, Pool for late; all emitted before the
           per-quad chains so the engine queues drain them during the
           DMA/AllReduce head).
  phase 2: per quad (16 j, [128, (s=4,z=16,k=32)] views):
           prod = u*t (DVE 2x, t bcast over s)
           z-tree 16->8->4->2 (Pool) -> lg f32 (Pool)
           e = exp(lg/4) (ACT); zq = sum_k e (DVE); rz (DVE); c = e*rz (DVE)
           prod2 = u*c (DVE 2x, c bcast over middle z)
           slot fold 4->2 (DVE) ; 2 ones-matmuls accumulate s in PSUM (PE)
  tail:    s_psum[32,512] -> SBUF -> DMA; host sums cores, squash.
"""

import sys

if "/opt/trn_rl_repo" not in sys.path:
    sys.path.insert(0, "/opt/trn_rl_repo")

import numpy as np
import os

B, N, D_IN = 32, 2048, 16
K, D_OUT = 32, 16
NCORES = 8
NS = N // NCORES          # 256 local j per core
NT = NS // 4              # 64 tiles of 4 j
NT2 = NT // 2             # 32 octets (8 j each)
NQ = NT2 // 2             # 16 quads (16 j each)
KZ = K * D_OUT            # 512
EPS = 1e-20

_CACHE = {}


def _pack_inputs(x, w, b):
    """Per-core host-side marshaling into the DMA-friendly layouts (bf16)."""
    import ml_dtypes
    bf = ml_dtypes.bfloat16
    xr = x.astype(bf).astype(np.float32)      # [B, N, D_IN]
    wr = w.astype(bf).astype(np.float32)      # [K, N, D_IN, D_OUT]
    ones_bd = np.tile(np.eye(B, dtype=np.float32), (4, 1)).astype(bf)  # [128, 32]
    per_core = []
    for r in range(NCORES):
        js, je = r * NS, (r + 1) * NS
        # w_host[64h+q, t2*512 + (z*32+k)] = w[k, js+(2*t2+h)*4+jr, i, z], q=jr*16+i
        wc = wr[:, js:je]                         # [K, NS, D_IN, D_OUT]
        wc = wc.transpose(1, 2, 3, 0)             # [NS, D_IN, D_OUT, K]  (j, i, z, k)
        wc = wc.reshape(NT, 64, KZ)               # [jt, (jr i), (z k)]
        wc = wc.reshape(NT2, 2, 64, KZ).transpose(1, 2, 0, 3)  # [h, q, t2, c]
        w_host = np.ascontiguousarray(wc.reshape(128, NT2 * KZ)).astype(bf)

        # block-diagonal x for the u matmuls (unchanged layout):
        # bdx[64h+q, t2*128 + jr*32 + b] = x[b, j(tile,jr), i] iff q == jr*16+i
        xc = xr[:, js:je, :]                      # [B, NS, D_IN]
        xc = xc.transpose(1, 2, 0)                # [NS, D_IN, B]  (j, i, b)
        bdx = np.zeros((2, 64, NT2, 128), dtype=np.float32)   # [h, q, t2, col]
        xt4 = xc.reshape(NT2, 2, 4, D_IN, B)      # [t2, h, jr, i, b]
        for jr in range(4):
            bdx[:, jr * 16:(jr + 1) * 16, :, jr * 32:(jr + 1) * 32] = (
                xt4[:, :, jr].transpose(1, 2, 0, 3)           # [h, i, t2, b]
            )
        bdx_host = np.ascontiguousarray(bdx.reshape(128, NT2 * 128)).astype(bf)

        # dense xT for the t matmuls: xt[jj*16+i, t2*32+b] = x[b, js+t2*8+jj, i]
        xt = xc.reshape(NT2, 8, D_IN, B)          # [t2, jj, i, b]
        xt = xt.transpose(1, 2, 0, 3)             # [jj, i, t2, b]
        xt_host = np.ascontiguousarray(xt.reshape(128, NT2 * B)).astype(bf)

        per_core.append({"w": w_host, "bdx": bdx_host, "xt": xt_host,
                         "ones": ones_bd})

    if np.any(b):
        # brep[p=(jr*32+bb), tile*K + k] = b[k, j(tile,jr)]  (replicated over bb)
        for r in range(NCORES):
            js = r * NS
            bc = b[:, js:js + NS, 0]                         # [K, NS]
            br = bc.transpose(1, 0).reshape(NT, 4, 1, K)     # [tile, jr, 1, k]
            br = np.broadcast_to(br, (NT, 4, 32, K))         # replicate over batch
            brep = br.transpose(1, 2, 0, 3).reshape(128, NT * K)
            per_core[r]["brep"] = np.ascontiguousarray(brep, dtype=np.float32)
        with_bias = True
    else:
        with_bias = False
    return per_core, with_bias


def _build(with_bias, sim_collective=False):
    from concourse import bacc, mybir
    from concourse.tile import TileContext

    f32 = mybir.dt.float32
    bf16d = mybir.dt.bfloat16

    nc = bacc.Bacc("TRN2", target_bir_lowering=False, debug=False,
                   num_devices=NCORES)
    w_in = nc.declare_dram_parameter("w", [128, NT2 * KZ], bf16d, isOutput=False)
    bdx_in = nc.declare_dram_parameter("bdx", [128, NT2 * 128], bf16d, isOutput=False)
    xt_in = nc.declare_dram_parameter("xt", [128, NT2 * B], bf16d, isOutput=False)
    ones_in = nc.declare_dram_parameter("ones", [128, B], bf16d, isOutput=False)
    brep_in = None
    if with_bias:
        brep_in = nc.declare_dram_parameter("brep", [128, NT * K], f32, isOutput=False)
    s_out = nc.declare_dram_parameter("s_part", [32, KZ], f32, isOutput=True)

    t_ar_in = nc.dram_tensor("t_ar_in", [32, KZ], bf16d)
    t_ar_out = nc.dram_tensor("t_ar_out", [32, KZ], bf16d, addr_space="Shared")

    with TileContext(nc) as tc:
        with (
            tc.tile_pool(name="wp", bufs=1) as wp,
            tc.tile_pool(name="xp", bufs=1) as xp,
            tc.tile_pool(name="ubp", bufs=1) as ubp,
            tc.tile_pool(name="sp", bufs=1) as sp,
            tc.tile_pool(name="work", bufs=2) as work,
            tc.tile_pool(name="zp", bufs=2) as zp,
            tc.tile_pool(name="small", bufs=4) as small,
            tc.tile_pool(name="pu", bufs=3, space="PSUM") as pu,
            tc.tile_pool(name="pt", bufs=1, space="PSUM") as pt,
            tc.tile_pool(name="psm", bufs=1, space="PSUM") as psm,
        ):
            bf16 = mybir.dt.bfloat16
            # ---- input DMAs: t-path first (xt, w), bdx interleaved so the
            # u-matmul/cast pipeline can start early; ones is tiny. ----
            w_sb = wp.tile([128, NT2 * KZ], bf16, tag="w")
            bdx_sb = xp.tile([128, NT2 * 128], bf16, tag="bdx")
            wq = NT2 * KZ // 8
            bq4 = NT2 * 128 // 4
            nc.sync.dma_start(out=w_sb[:, 0:wq], in_=w_in[:, 0:wq])
            xt_sb = xp.tile([128, NT2 * B], bf16, tag="xt")
            nc.sync.dma_start(out=xt_sb[:, :], in_=xt_in[:, :])
            ones_sb = xp.tile([128, B], bf16, tag="ones")
            nc.sync.dma_start(out=ones_sb[:, :], in_=ones_in[:, :])
            nc.sync.dma_start(out=bdx_sb[:, 0:bq4], in_=bdx_in[:, 0:bq4])
            for ci in range(1, 8):
                nc.sync.dma_start(out=w_sb[:, ci * wq:(ci + 1) * wq],
                                  in_=w_in[:, ci * wq:(ci + 1) * wq])

            brep_sb = None
            if with_bias:
                brep_sb = xp.tile([128, NT * K], f32, tag="brep")
                nc.sync.dma_start(out=brep_sb[:, :], in_=brep_in[:, :])

            # ---- phase 1: partial t ([32, (z,k)] psum, accumulated);
            # u-matmuls+casts for the first 8 octets ride along so the ACT
            # queue has cast work during the head (PE keeps up at full
            # p-state). ----
            u_bf = ubp.tile([128, NT2 * 1024], bf16, tag="u_bf")

            def emit_octet(o, cast_eng):
                uo = pu.tile([128, 1024], f32, tag="u")
                for h in range(2):
                    nc.tensor.matmul(uo[:, h * KZ:(h + 1) * KZ],
                                     bdx_sb[64 * h:64 * h + 64,
                                            o * 128:(o + 1) * 128],
                                     w_sb[64 * h:64 * h + 64,
                                          o * KZ:(o + 1) * KZ],
                                     start=True, stop=True)
                # GPSIMD cannot access PSUM on real HW: casts are ACT-only
                nc.scalar.copy(u_bf[:, o * 1024:(o + 1) * 1024], uo[:, :])

            t_ps = pt.tile([32, KZ], f32, tag="t")
            for t2 in range(NT2):
                nc.tensor.matmul(t_ps[0:32, 0:KZ],
                                 xt_sb[:, t2 * B:(t2 + 1) * B],
                                 w_sb[:, t2 * KZ:(t2 + 1) * KZ],
                                 start=(t2 == 0), stop=(t2 == NT2 - 1))
                if t2 < 8:
                    emit_octet(t2, "act")
            # t_loc copy on DVE (idle pre-AllReduce; ACT stays free for
            # the u casts); casts straight to bf16 so the AllReduce ships
            # half the bytes and the post-AR cast disappears.
            t_loc = sp.tile([32, KZ], bf16, tag="t_loc")
            nc.vector.tensor_copy(t_loc[:, :], t_ps[0:32, 0:KZ])
            nc.sync.dma_start(out=t_ar_in[:, :], in_=t_loc[:, :])
            if sim_collective:
                # timing-model stand-in for the AllReduce (single-core sim)
                nc.gpsimd.dma_start(out=t_ar_out[:, :], in_=t_ar_in[:, :])
            else:
                nc.gpsimd.collective_compute(
                    "AllReduce",
                    mybir.AluOpType.add,
                    replica_groups=[list(range(NCORES))],
                    ins=[t_ar_in[:, :].opt()],
                    outs=[t_ar_out[:, :].opt()],
                )
            # late bdx quarters ride the window left idle by the AllReduce
            for ci in range(1, 4):
                nc.sync.dma_start(out=bdx_sb[:, ci * bq4:(ci + 1) * bq4],
                                  in_=bdx_in[:, ci * bq4:(ci + 1) * bq4])
            # replicate t to all 4 jr partition groups: one DMA + partition
            # doubling (DVE is idle at this point; a single broadcast-read
            # DMA would degenerate to per-element descriptors)
            t_rep_bf = sp.tile([128, KZ], bf16, tag="t_rep_bf")
            nc.sync.dma_start(out=t_rep_bf[0:32, :], in_=t_ar_out[:, :])
            nc.vector.tensor_copy(t_rep_bf[32:64, :], t_rep_bf[0:32, :])
            nc.vector.tensor_copy(t_rep_bf[64:128, :], t_rep_bf[0:64, :])

            t_bc = (t_rep_bf[:, :]
                    .rearrange("p (z k) -> p z k", k=K)
                    .unsqueeze(1).to_broadcast((128, 4, D_OUT, K)))

            s_ps = psm.tile([32, KZ], f32, tag="s")

            # ---- phase 2: per quad; u-matmuls+casts for octets 8..31
            # stream through the loop (ACT for 8..23, Pool for 24..31),
            # always >= 4 quads ahead of their consumer ----
            for q in range(NQ):
                for o in (8 + 2 * q, 9 + 2 * q):
                    if o < NT2:
                        emit_octet(o, "act")
                uq = (u_bf[:, q * 2048:(q + 1) * 2048]
                      .rearrange("p (s z k) -> p s z k", s=4, k=K))
                # prod = u * t  (DVE 2x; t broadcast over the 4 slots)
                P1 = work.tile([128, 2048], bf16, tag="P1")
                nc.vector.tensor_tensor(
                    P1[:, :].rearrange("p (s z k) -> p s z k", s=4, k=K),
                    uq, t_bc, op=mybir.AluOpType.mult)
                # z-tree 16->8->4->2 on Pool (k stays innermost unit stride)
                p1v = P1[:, :].rearrange("p (s z k) -> p s z k", s=4, k=K)
                Z1 = zp.tile([128, 1024], bf16, tag="Z1")
                z1v = Z1[:, :].rearrange("p (s z k) -> p s z k", s=4, k=K)
                nc.gpsimd.tensor_tensor(z1v, p1v[:, :, 0:8], p1v[:, :, 8:16],
                                        op=mybir.AluOpType.add)
                Z2 = zp.tile([128, 512], bf16, tag="Z2")
                z2v = Z2[:, :].rearrange("p (s z k) -> p s z k", s=4, k=K)
                nc.vector.tensor_tensor(z2v, z1v[:, :, 0:4], z1v[:, :, 4:8],
                                        op=mybir.AluOpType.add)
                Z3 = zp.tile([128, 256], bf16, tag="Z3")
                z3v = Z3[:, :].rearrange("p (s z k) -> p s z k", s=4, k=K)
                nc.gpsimd.tensor_tensor(z3v, z2v[:, :, 0:2], z2v[:, :, 2:4],
                                        op=mybir.AluOpType.add)
                lg = small.tile([128, 4 * K], f32, tag="lg")
                nc.gpsimd.tensor_tensor(
                    lg[:, :].rearrange("p (s k) -> p s k", k=K),
                    z3v[:, :, 0, :], z3v[:, :, 1, :], op=mybir.AluOpType.add)
                # softmax over k (per slot)
                e = small.tile([128, 4 * K], f32, tag="e")
                nc.scalar.activation(e[:, :], lg[:, :],
                                     mybir.ActivationFunctionType.Exp,
                                     scale=0.25)
                zq = small.tile([128, 4], f32, tag="zq")
                nc.vector.tensor_reduce(
                    zq[:, :],
                    e[:, :].rearrange("p (s k) -> p s k", k=K),
                    axis=mybir.AxisListType.X, op=mybir.AluOpType.add)
                rz = small.tile([128, 4], f32, tag="rz")
                nc.vector.reciprocal(rz[:, :], zq[:, :])
                c = small.tile([128, 4 * K], bf16, tag="c")
                rz_bc = rz[:, :].unsqueeze(-1).to_broadcast((128, 4, K))
                nc.gpsimd.tensor_tensor(
                    c[:, :].rearrange("p (s k) -> p s k", k=K),
                    e[:, :].rearrange("p (s k) -> p s k", k=K),
                    rz_bc, op=mybir.AluOpType.mult)
                if with_bias:
                    nc.vector.tensor_tensor(
                        c[:, :], c[:, :],
                        brep_sb[:, q * 4 * K:(q + 1) * 4 * K],
                        op=mybir.AluOpType.add)

                # prod2 = u * c (c broadcast over the middle z dim); slots
                # 0-2 on DVE (2x), slot 3 on Pool to balance the engines
                c_bc3 = (c[:, 0:3 * K].rearrange("p (s k) -> p s k", k=K)
                         .unsqueeze(2).to_broadcast((128, 3, D_OUT, K)))
                P2 = work.tile([128, 2048], bf16, tag="P2")
                nc.vector.tensor_tensor(
                    P2[:, 0:1536].rearrange("p (s z k) -> p s z k", s=3, k=K),
                    uq[:, 0:3], c_bc3, op=mybir.AluOpType.mult)
                c_s3 = (c[:, 3 * K:4 * K].unsqueeze(1)
                        .to_broadcast((128, D_OUT, K)))
                nc.gpsimd.tensor_tensor(
                    P2[:, 1536:2048].rearrange("p (z k) -> p z k", k=K),
                    uq[:, 3], c_s3, op=mybir.AluOpType.mult)
                # slot fold 4->2 (DVE 2x), then two PE ones-matmuls
                # accumulate s[b,(z,k)] in PSUM across all quads
                SH = work.tile([128, 1024], bf16, tag="SH")
                p2v = P2[:, :].rearrange("p (s c) -> p s c", s=4)
                nc.gpsimd.tensor_tensor(
                    SH[:, :].rearrange("p (s c) -> p s c", s=2),
                    p2v[:, 0:2], p2v[:, 2:4], op=mybir.AluOpType.add)
                nc.tensor.matmul(s_ps[0:32, 0:KZ], ones_sb[:, 0:B],
                                 SH[:, 0:KZ],
                                 start=(q == 0), stop=False)
                nc.tensor.matmul(s_ps[0:32, 0:KZ], ones_sb[:, 0:B],
                                 SH[:, KZ:2 * KZ],
                                 start=False, stop=(q == NQ - 1))

            # ---- tail ----
            s_sb = sp.tile([32, KZ], f32, tag="s_sb")
            nc.scalar.copy(s_sb[:, :], s_ps[0:32, 0:KZ])
            nc.sync.dma_start(out=s_out[:, :], in_=s_sb[:, :])

    nc.compile()
    return nc


def _get_nc(with_bias):
    key = ("nc", with_bias)
    if key not in _CACHE:
        _CACHE[key] = _build(with_bias)
    return _CACHE[key]


def _get_runner(with_bias):
    """Build (once) a cached shard_map-jitted executable for the 8-core SPMD
    kernel, mirroring bass2jax.run_bass_via_pjrt but reusable across calls."""
    key = ("runner", with_bias)
    if key in _CACHE:
        return _CACHE[key]

    import jax
    from jax.sharding import Mesh, PartitionSpec
    from jax.experimental.shard_map import shard_map
    from concourse import mybir
    from concourse import bass2jax
    from concourse.bass2jax import (_bass_exec_p, install_neuronx_cc_hook,
                                    partition_id_tensor)

    install_neuronx_cc_hook()
    nc = _get_nc(with_bias)

    partition_name = nc.partition_id_tensor.name if nc.partition_id_tensor else None
    in_names, out_names, out_avals, zero_shapes = [], [], [], []
    for alloc in nc.m.functions[0].allocations:
        if not isinstance(alloc, mybir.MemoryLocationSet):
            continue
        name = alloc.memorylocations[0].name
        if alloc.kind == "ExternalInput":
            if name != partition_name:
                in_names.append(name)
        elif alloc.kind == "ExternalOutput":
            out_names.append(name)
            shape = tuple(alloc.tensor_shape)
            dtype = mybir.dt.np(alloc.dtype)
            out_avals.append(jax.core.ShapedArray(shape, dtype))
            zero_shapes.append((shape, dtype))
    n_params = len(in_names)
    n_outs = len(out_avals)
    all_in_names = list(in_names) + list(out_names)
    if partition_name is not None:
        all_in_names.append(partition_name)

    def _body(*args):
        operands = list(args)
        if partition_name is not None:
            operands.append(partition_id_tensor())
        outs = _bass_exec_p.bind(
            *operands,
            out_avals=tuple(out_avals),
            in_names=tuple(all_in_names),
            out_names=tuple(out_names),
            lowering_input_output_aliases=(),
            sim_require_finite=True,
            sim_require_nnan=True,
            nc=nc,
        )
        return tuple(outs)

    devices = jax.devices()[:NCORES]
    mesh = Mesh(np.asarray(devices), ("core",))
    in_specs = (PartitionSpec("core"),) * (n_params + n_outs)
    out_specs = (PartitionSpec("core"),) * n_outs
    donate = tuple(range(n_params, n_params + n_outs))
    sharded = jax.jit(
        shard_map(_body, mesh=mesh, in_specs=in_specs, out_specs=out_specs,
                  check_rep=False),
        donate_argnums=donate, keep_unused=True)

    def run(per_core):
        concat_in = [
            np.concatenate([np.asarray(per_core[c][nm]) for c in range(NCORES)], axis=0)
            for nm in in_names
        ]
        concat_zeros = [np.zeros((NCORES * sh[0], *sh[1:]), dt)
                        for sh, dt in zero_shapes]
        out_arrs = sharded(*concat_in, *concat_zeros)
        return [
            {nm: np.asarray(out_arrs[i]).reshape(NCORES, *out_avals[i].shape)[c]
             for i, nm in enumerate(out_names)}
            for c in range(NCORES)
        ]

    _CACHE[key] = run
    return run


def kernel(x, w, b, _run_kwargs=None):
    x = np.asarray(x, dtype=np.float32)
    w = np.asarray(w, dtype=np.float32)
    b = np.asarray(b, dtype=np.float32)

    per_core, with_bias = _pack_inputs(x, w, b)
    results = _get_runner(with_bias)(per_core)

    s = np.zeros((B, KZ), dtype=np.float64)
    for r in range(NCORES):
        s += results[r]["s_part"].astype(np.float64)
    # cols are (z,k): s[b, z*32+k] -> [B, K, D_OUT]
    s = s.reshape(B, D_OUT, K).transpose(0, 2, 1)
    s = np.ascontiguousarray(s).astype(np.float32)

    # efficient squash (host-side finalization of the gathered partials)
    n = np.linalg.norm(s.astype(np.float64), axis=-1, keepdims=True)
    out = (1.0 - 1.0 / (np.exp(n) + EPS)) * (s / (n + EPS))
    return out.astype(np.float32)


# revision 26
# speedup vs baseline: 1.0657x; 1.0066x over previous
"""Trainium2 Bass kernel for nn_Capsule (Efficient-CapsNet style capsule layer).

Math (see reference):
    u[b,k,j,:] = x[b,j,:] @ w[k,j,:,:]            # per-(k,j) 16x16 projection
    t[b,k,:]   = sum_j u[b,k,j,:]
    l[b,k,j]   = <u[b,k,j,:], t[b,k,:]> / sqrt(D)
    c          = softmax_k(l) + bias
    s[b,k,:]   = sum_j c[b,k,j] u[b,k,j,:]
    out        = squash(s)

Sharding: the j (N=2048) contraction axis is split over 8 cores (256 j each),
so each core reads only its w slice once (4 MB in bf16).  Cross-core coupling
is a single 64 KB AllReduce of t; the softmax over k is core-local.  Per-core
partial s [32,512] are summed on host, followed by the (tiny) squash.

Key layout trick vs the previous version: w columns are packed (z,k) with k
innermost (col = z*32 + k).  Then the softmax weights c (which vary over k
but not z) broadcast over the MIDDLE z dim of a [p,(s,z,k)] view, keeping
unit stride innermost, so the c-apply multiply runs at DVE 2x with NO
materialized c replication.  The s j-reduction runs on the PE as a
block-diagonal ones matmul accumulating into PSUM across all quads.

Per-core schedule:
  DMA:     xt+ones first, then w (z,k)-packed + bdx interleaved.
  phase 1: 32 accumulating bf16 t-matmuls -> t_partial[32,(z,k)]
           -> AllReduce(t) -> replicate to t_rep[128,512] + bf16 cast
  u-path:  per octet (8 j): 2 matmuls -> PSUM [128,1024] -> cast to SBUF
           bf16 (ACT for early octets, Pool for late; all emitted before the
           per-quad chains so the engine queues drain them during the
           DMA/AllReduce head).
  phase 2: per quad (16 j, [128, (s=4,z=16,k=32)] views):
           prod = u*t (DVE 2x, t bcast over s)
           z-tree 16->8->4->2 (Pool) -> lg f32 (Pool)
           e = exp(lg/4) (ACT); zq = sum_k e (DVE); rz (DVE); c = e*rz (DVE)
           prod2 = u*c (DVE 2x, c bcast over middle z)
           slot fold 4->2 (DVE) ; 2 ones-matmuls accumulate s in PSUM (PE)
  tail:    s_psum[32,512] -> SBUF -> DMA; host sums cores, squash.
"""

import sys

if "/opt/trn_rl_repo" not in sys.path:
    sys.path.insert(0, "/opt/trn_rl_repo")

import numpy as np
import os

B, N, D_IN = 32, 2048, 16
K, D_OUT = 32, 16
NCORES = 8
NS = N // NCORES          # 256 local j per core
NT = NS // 4              # 64 tiles of 4 j
NT2 = NT // 2             # 32 octets (8 j each)
NQ = NT2 // 2             # 16 quads (16 j each)
KZ = K * D_OUT            # 512
EPS = 1e-20

_CACHE = {}


def _pack_inputs(x, w, b):
    """Per-core host-side marshaling into the DMA-friendly layouts (bf16)."""
    import ml_dtypes
    bf = ml_dtypes.bfloat16
    xr = x.astype(bf).astype(np.float32)      # [B, N, D_IN]
    wr = w.astype(bf).astype(np.float32)      # [K, N, D_IN, D_OUT]
    ones_bd = np.tile(np.eye(B, dtype=np.float32), (4, 1)).astype(bf)  # [128, 32]
    per_core = []
    for r in range(NCORES):
        js, je = r * NS, (r + 1) * NS
        # w_host[64h+q, t2*512 + (z*32+k)] = w[k, js+(2*t2+h)*4+jr, i, z], q=jr*16+i
        wc = wr[:, js:je]                         # [K, NS, D_IN, D_OUT]
        wc = wc.transpose(1, 2, 3, 0)             # [NS, D_IN, D_OUT, K]  (j, i, z, k)
        wc = wc.reshape(NT, 64, KZ)               # [jt, (jr i), (z k)]
        wc = wc.reshape(NT2, 2, 64, KZ).transpose(1, 2, 0, 3)  # [h, q, t2, c]
        w_host = np.ascontiguousarray(wc.reshape(128, NT2 * KZ)).astype(bf)

        # block-diagonal x for the u matmuls (unchanged layout):
        # bdx[64h+q, t2*128 + jr*32 + b] = x[b, j(tile,jr), i] iff q == jr*16+i
        xc = xr[:, js:je, :]                      # [B, NS, D_IN]
        xc = xc.transpose(1, 2, 0)                # [NS, D_IN, B]  (j, i, b)
        bdx = np.zeros((2, 64, NT2, 128), dtype=np.float32)   # [h, q, t2, col]
        xt4 = xc.reshape(NT2, 2, 4, D_IN, B)      # [t2, h, jr, i, b]
        for jr in range(4):
            bdx[:, jr * 16:(jr + 1) * 16, :, jr * 32:(jr + 1) * 32] = (
                xt4[:, :, jr].transpose(1, 2, 0, 3)           # [h, i, t2, b]
            )
        bdx_host = np.ascontiguousarray(bdx.reshape(128, NT2 * 128)).astype(bf)

        # dense xT for the t matmuls: xt[jj*16+i, t2*32+b] = x[b, js+t2*8+jj, i]
        xt = xc.reshape(NT2, 8, D_IN, B)          # [t2, jj, i, b]
        xt = xt.transpose(1, 2, 0, 3)             # [jj, i, t2, b]
        xt_host = np.ascontiguousarray(xt.reshape(128, NT2 * B)).astype(bf)

        per_core.append({"w": w_host, "bdx": bdx_host, "xt": xt_host,
                         "ones": ones_bd})

    if np.any(b):
        # brep[p=(jr*32+bb), tile*K + k] = b[k, j(tile,jr)]  (replicated over bb)
        for r in range(NCORES):
            js = r * NS
            bc = b[:, js:js + NS, 0]                         # [K, NS]
            br = bc.transpose(1, 0).reshape(NT, 4, 1, K)     # [tile, jr, 1, k]
            br = np.broadcast_to(br, (NT, 4, 32, K))         # replicate over batch
            brep = br.transpose(1, 2, 0, 3).reshape(128, NT * K)
            per_core[r]["brep"] = np.ascontiguousarray(brep, dtype=np.float32)
        with_bias = True
    else:
        with_bias = False
    return per_core, with_bias


def _build(with_bias, sim_collective=False):
    from concourse import bacc, mybir
    from concourse.tile import TileContext

    f32 = mybir.dt.float32
    bf16d = mybir.dt.bfloat16

    nc = bacc.Bacc("TRN2", target_bir_lowering=False, debug=False,
                   num_devices=NCORES)
    w_in = nc.declare_dram_parameter("w", [128, NT2 * KZ], bf16d, isOutput=False)
    bdx_in = nc.declare_dram_parameter("bdx", [128, NT2 * 128], bf16d, isOutput=False)
    xt_in = nc.declare_dram_parameter("xt", [128, NT2 * B], bf16d, isOutput=False)
    ones_in = nc.declare_dram_parameter("ones", [128, B], bf16d, isOutput=False)
    brep_in = None
    if with_bias:
        brep_in = nc.declare_dram_parameter("brep", [128, NT * K], f32, isOutput=False)
    s_out = nc.declare_dram_parameter("s_part", [32, KZ], f32, isOutput=True)

    t_ar_in = nc.dram_tensor("t_ar_in", [32, KZ], bf16d)
    t_ar_out = nc.dram_tensor("t_ar_out", [32, KZ], bf16d, addr_space="Shared")

    with TileContext(nc) as tc:
        with (
            tc.tile_pool(name="wp", bufs=1) as wp,
            tc.tile_pool(name="xp", bufs=1) as xp,
            tc.tile_pool(name="ubp", bufs=1) as ubp,
            tc.tile_pool(name="sp", bufs=1) as sp,
            tc.tile_pool(name="work", bufs=2) as work,
            tc.tile_pool(name="zp", bufs=2) as zp,
            tc.tile_pool(name="small", bufs=4) as small,
            tc.tile_pool(name="pu", bufs=3, space="PSUM") as pu,
            tc.tile_pool(name="pt", bufs=1, space="PSUM") as pt,
            tc.tile_pool(name="psm", bufs=1, space="PSUM") as psm,
        ):
            bf16 = mybir.dt.bfloat16
            # ---- input DMAs: t-path first (xt, w), bdx interleaved so the
            # u-matmul/cast pipeline can start early; ones is tiny. ----
            w_sb = wp.tile([128, NT2 * KZ], bf16, tag="w")
            bdx_sb = xp.tile([128, NT2 * 128], bf16, tag="bdx")
            wq = NT2 * KZ // 8
            bq4 = NT2 * 128 // 4
            nc.sync.dma_start(out=w_sb[:, 0:wq], in_=w_in[:, 0:wq])
            xt_sb = xp.tile([128, NT2 * B], bf16, tag="xt")
            nc.sync.dma_start(out=xt_sb[:, :], in_=xt_in[:, :])
            for ci in range(1, 8):
                nc.sync.dma_start(out=w_sb[:, ci * wq:(ci + 1) * wq],
                                  in_=w_in[:, ci * wq:(ci + 1) * wq])
            ones_sb = xp.tile([128, B], bf16, tag="ones")
            nc.sync.dma_start(out=ones_sb[:, :], in_=ones_in[:, :])
            nc.sync.dma_start(out=bdx_sb[:, 0:bq4], in_=bdx_in[:, 0:bq4])

            brep_sb = None
            if with_bias:
                brep_sb = xp.tile([128, NT * K], f32, tag="brep")
                nc.sync.dma_start(out=brep_sb[:, :], in_=brep_in[:, :])

            # ---- phase 1: partial t ([32, (z,k)] psum, accumulated);
            # u-matmuls+casts for the first 8 octets ride along so the ACT
            # queue has cast work during the head (PE keeps up at full
            # p-state). ----
            u_bf = ubp.tile([128, NT2 * 1024], bf16, tag="u_bf")

            def emit_octet(o, cast_eng):
                uo = pu.tile([128, 1024], f32, tag="u")
                for h in range(2):
                    nc.tensor.matmul(uo[:, h * KZ:(h + 1) * KZ],
                                     bdx_sb[64 * h:64 * h + 64,
                                            o * 128:(o + 1) * 128],
                                     w_sb[64 * h:64 * h + 64,
                                          o * KZ:(o + 1) * KZ],
                                     start=True, stop=True)
                # GPSIMD cannot access PSUM on real HW: casts are ACT-only
                nc.scalar.copy(u_bf[:, o * 1024:(o + 1) * 1024], uo[:, :])

            t_ps = pt.tile([32, KZ], f32, tag="t")
            for t2 in range(NT2):
                nc.tensor.matmul(t_ps[0:32, 0:KZ],
                                 xt_sb[:, t2 * B:(t2 + 1) * B],
                                 w_sb[:, t2 * KZ:(t2 + 1) * KZ],
                                 start=(t2 == 0), stop=(t2 == NT2 - 1))
                if t2 < 8:
                    emit_octet(t2, "act")
            # t_loc copy on DVE (idle pre-AllReduce; ACT stays free for
            # the u casts); casts straight to bf16 so the AllReduce ships
            # half the bytes and the post-AR cast disappears.
            t_loc = sp.tile([32, KZ], bf16, tag="t_loc")
            nc.vector.tensor_copy(t_loc[:, :], t_ps[0:32, 0:KZ])
            nc.sync.dma_start(out=t_ar_in[:, :], in_=t_loc[:, :])
            if sim_collective:
                # timing-model stand-in for the AllReduce (single-core sim)
                nc.gpsimd.dma_start(out=t_ar_out[:, :], in_=t_ar_in[:, :])
            else:
                nc.gpsimd.collective_compute(
                    "AllReduce",
                    mybir.AluOpType.add,
                    replica_groups=[list(range(NCORES))],
                    ins=[t_ar_in[:, :].opt()],
                    outs=[t_ar_out[:, :].opt()],
                )
            # late bdx quarters ride the window left idle by the AllReduce
            for ci in range(1, 4):
                nc.sync.dma_start(out=bdx_sb[:, ci * bq4:(ci + 1) * bq4],
                                  in_=bdx_in[:, ci * bq4:(ci + 1) * bq4])
            # replicate t to all 4 jr partition groups: one DMA + partition
            # doubling (DVE is idle at this point; a single broadcast-read
            # DMA would degenerate to per-element descriptors)
            t_rep_bf = sp.tile([128, KZ], bf16, tag="t_rep_bf")
            nc.sync.dma_start(out=t_rep_bf[0:32, :], in_=t_ar_out[:, :])
            nc.vector.tensor_copy(t_rep_bf[32:64, :], t_rep_bf[0:32, :])
            nc.vector.tensor_copy(t_rep_bf[64:128, :], t_rep_bf[0:64, :])

            t_bc = (t_rep_bf[:, :]
                    .rearrange("p (z k) -> p z k", k=K)
                    .unsqueeze(1).to_broadcast((128, 4, D_OUT, K)))

            s_ps = psm.tile([32, KZ], f32, tag="s")

            # ---- phase 2: per quad; u-matmuls+casts for octets 8..31
            # stream through the loop (ACT for 8..23, Pool for 24..31),
            # always >= 4 quads ahead of their consumer ----
            for q in range(NQ):
                for o in (8 + 2 * q, 9 + 2 * q):
                    if o < NT2:
                        emit_octet(o, "act")
                uq = (u_bf[:, q * 2048:(q + 1) * 2048]
                      .rearrange("p (s z k) -> p s z k", s=4, k=K))
                # prod = u * t  (DVE 2x; t broadcast over the 4 slots)
                P1 = work.tile([128, 2048], bf16, tag="P1")
                nc.vector.tensor_tensor(
                    P1[:, :].rearrange("p (s z k) -> p s z k", s=4, k=K),
                    uq, t_bc, op=mybir.AluOpType.mult)
                # z-tree 16->8->4->2 on Pool (k stays innermost unit stride)
                p1v = P1[:, :].rearrange("p (s z k) -> p s z k", s=4, k=K)
                Z1 = zp.tile([128, 1024], bf16, tag="Z1")
                z1v = Z1[:, :].rearrange("p (s z k) -> p s z k", s=4, k=K)
                nc.gpsimd.tensor_tensor(z1v, p1v[:, :, 0:8], p1v[:, :, 8:16],
                                        op=mybir.AluOpType.add)
                Z2 = zp.tile([128, 512], bf16, tag="Z2")
                z2v = Z2[:, :].rearrange("p (s z k) -> p s z k", s=4, k=K)
                nc.vector.tensor_tensor(z2v, z1v[:, :, 0:4], z1v[:, :, 4:8],
                                        op=mybir.AluOpType.add)
                Z3 = zp.tile([128, 256], bf16, tag="Z3")
                z3v = Z3[:, :].rearrange("p (s z k) -> p s z k", s=4, k=K)
                nc.gpsimd.tensor_tensor(z3v, z2v[:, :, 0:2], z2v[:, :, 2:4],
                                        op=mybir.AluOpType.add)
                lg = small.tile([128, 4 * K], f32, tag="lg")
                nc.gpsimd.tensor_tensor(
                    lg[:, :].rearrange("p (s k) -> p s k", k=K),
                    z3v[:, :, 0, :], z3v[:, :, 1, :], op=mybir.AluOpType.add)
                # softmax over k (per slot)
                e = small.tile([128, 4 * K], f32, tag="e")
                nc.scalar.activation(e[:, :], lg[:, :],
                                     mybir.ActivationFunctionType.Exp,
                                     scale=0.25)
                zq = small.tile([128, 4], f32, tag="zq")
                nc.vector.tensor_reduce(
                    zq[:, :],
                    e[:, :].rearrange("p (s k) -> p s k", k=K),
                    axis=mybir.AxisListType.X, op=mybir.AluOpType.add)
                rz = small.tile([128, 4], f32, tag="rz")
                nc.vector.reciprocal(rz[:, :], zq[:, :])
                c = small.tile([128, 4 * K], bf16, tag="c")
                rz_bc = rz[:, :].unsqueeze(-1).to_broadcast((128, 4, K))
                nc.gpsimd.tensor_tensor(
                    c[:, :].rearrange("p (s k) -> p s k", k=K),
                    e[:, :].rearrange("p (s k) -> p s k", k=K),
                    rz_bc, op=mybir.AluOpType.mult)
                if with_bias:
                    nc.vector.tensor_tensor(
                        c[:, :], c[:, :],
                        brep_sb[:, q * 4 * K:(q + 1) * 4 * K],
                        op=mybir.AluOpType.add)

                # prod2 = u * c (c broadcast over the middle z dim); slots
                # 0-2 on DVE (2x), slot 3 on Pool to balance the engines
                c_bc3 = (c[:, 0:3 * K].rearrange("p (s k) -> p s k", k=K)
                         .unsqueeze(2).to_broadcast((128, 3, D_OUT, K)))
                P2 = work.tile([128, 2048], bf16, tag="P2")
                nc.vector.tensor_tensor(
                    P2[:, 0:1536].rearrange("p (s z k) -> p s z k", s=3, k=K),
                    uq[:, 0:3], c_bc3, op=mybir.AluOpType.mult)
                c_s3 = (c[:, 3 * K:4 * K].unsqueeze(1)
                        .to_broadcast((128, D_OUT, K)))
                nc.gpsimd.tensor_tensor(
                    P2[:, 1536:2048].rearrange("p (z k) -> p z k", k=K),
                    uq[:, 3], c_s3, op=mybir.AluOpType.mult)
                # slot fold 4->2 (DVE 2x), then two PE ones-matmuls
                # accumulate s[b,(z,k)] in PSUM across all quads
                SH = work.tile([128, 1024], bf16, tag="SH")
                p2v = P2[:, :].rearrange("p (s c) -> p s c", s=4)
                nc.gpsimd.tensor_tensor(
                    SH[:, :].rearrange("p (s c) -> p s c", s=2),
                    p2v[:, 0:2], p2v[:, 2:4], op=mybir.AluOpType.add)
                nc.tensor.matmul(s_ps[0:32, 0:KZ], ones_sb[:, 0:B],
                                 SH[:, 0:KZ],
                                 start=(q == 0), stop=False)
                nc.tensor.matmul(s_ps[0:32, 0:KZ], ones_sb[:, 0:B],
                                 SH[:, KZ:2 * KZ],
                                 start=False, stop=(q == NQ - 1))

            # ---- tail ----
            s_sb = sp.tile([32, KZ], f32, tag="s_sb")
            nc.scalar.copy(s_sb[:, :], s_ps[0:32, 0:KZ])
            nc.sync.dma_start(out=s_out[:, :], in_=s_sb[:, :])

    nc.compile()
    return nc


def _get_nc(with_bias):
    key = ("nc", with_bias)
    if key not in _CACHE:
        _CACHE[key] = _build(with_bias)
    return _CACHE[key]


def _get_runner(with_bias):
    """Build (once) a cached shard_map-jitted executable for the 8-core SPMD
    kernel, mirroring bass2jax.run_bass_via_pjrt but reusable across calls."""
    key = ("runner", with_bias)
    if key in _CACHE:
        return _CACHE[key]

    import jax
    from jax.sharding import Mesh, PartitionSpec
    from jax.experimental.shard_map import shard_map
    from concourse import mybir
    from concourse import bass2jax
    from concourse.bass2jax import (_bass_exec_p, install_neuronx_cc_hook,
                                    partition_id_tensor)

    install_neuronx_cc_hook()
    nc = _get_nc(with_bias)

    partition_name = nc.partition_id_tensor.name if nc.partition_id_tensor else None
    in_names, out_names, out_avals, zero_shapes = [], [], [], []
    for alloc in nc.m.functions[0].allocations:
        if not isinstance(alloc, mybir.MemoryLocationSet):
            continue
        name = alloc.memorylocations[0].name
        if alloc.kind == "ExternalInput":
            if name != partition_name:
                in_names.append(name)
        elif alloc.kind == "ExternalOutput":
            out_names.append(name)
            shape = tuple(alloc.tensor_shape)
            dtype = mybir.dt.np(alloc.dtype)
            out_avals.append(jax.core.ShapedArray(shape, dtype))
            zero_shapes.append((shape, dtype))
    n_params = len(in_names)
    n_outs = len(out_avals)
    all_in_names = list(in_names) + list(out_names)
    if partition_name is not None:
        all_in_names.append(partition_name)

    def _body(*args):
        operands = list(args)
        if partition_name is not None:
            operands.append(partition_id_tensor())
        outs = _bass_exec_p.bind(
            *operands,
            out_avals=tuple(out_avals),
            in_names=tuple(all_in_names),
            out_names=tuple(out_names),
            lowering_input_output_aliases=(),
            sim_require_finite=True,
            sim_require_nnan=True,
            nc=nc,
        )
        return tuple(outs)

    devices = jax.devices()[:NCORES]
    mesh = Mesh(np.asarray(devices), ("core",))
    in_specs = (PartitionSpec("core"),) * (n_params + n_outs)
    out_specs = (PartitionSpec("core"),) * n_outs
    donate = tuple(range(n_params, n_params + n_outs))
    sharded = jax.jit(
        shard_map(_body, mesh=mesh, in_specs=in_specs, out_specs=out_specs,
                  check_rep=False),
        donate_argnums=donate, keep_unused=True)

    def run(per_core):
        concat_in = [
            np.concatenate([np.asarray(per_core[c][nm]) for c in range(NCORES)], axis=0)
            for nm in in_names
        ]
        concat_zeros = [np.zeros((NCORES * sh[0], *sh[1:]), dt)
                        for sh, dt in zero_shapes]
        out_arrs = sharded(*concat_in, *concat_zeros)
        return [
            {nm: np.asarray(out_arrs[i]).reshape(NCORES, *out_avals[i].shape)[c]
             for i, nm in enumerate(out_names)}
            for c in range(NCORES)
        ]

    _CACHE[key] = run
    return run


def kernel(x, w, b, _run_kwargs=None):
    x = np.asarray(x, dtype=np.float32)
    w = np.asarray(w, dtype=np.float32)
    b = np.asarray(b, dtype=np.float32)

    per_core, with_bias = _pack_inputs(x, w, b)
    results = _get_runner(with_bias)(per_core)

    s = np.zeros((B, KZ), dtype=np.float64)
    for r in range(NCORES):
        s += results[r]["s_part"].astype(np.float64)
    # cols are (z,k): s[b, z*32+k] -> [B, K, D_OUT]
    s = s.reshape(B, D_OUT, K).transpose(0, 2, 1)
    s = np.ascontiguousarray(s).astype(np.float32)

    # efficient squash (host-side finalization of the gathered partials)
    n = np.linalg.norm(s.astype(np.float64), axis=-1, keepdims=True)
    out = (1.0 - 1.0 / (np.exp(n) + EPS)) * (s / (n + EPS))
    return out.astype(np.float32)
